# revision 19
# baseline (speedup 1.0000x reference)
"""Trainium2 Bass kernel for nn_AblationLayer.

Reference semantics (B=32, C=1024, H=W=56):
    m0 = min(x)                              # global min over all elements
    vals[i] = 0           if m0 == 0
            = m0 - (i+1)*1e7  otherwise      # i = batch index
    out = x;  out[i, indices[i], :, :] = vals[i]

Strategy: data-parallel over batch across 8 NeuronCores (4 batch items
per core).  Each core streams its 51.4MB shard through SBUF once,
computing a running min on the Vector engine while copying the data to
the output (fused min+copy: one HBM read + one HBM write).  The eight
local minima are exchanged with a tiny AllGather; each core reduces
them to the global min, computes its 4 scatter values, and overwrites
the 4 target rows with one indirect (dynamically addressed) DMA.
"""

import sys

import numpy as np

if "/opt/trn_rl_repo" not in sys.path:
    sys.path.insert(0, "/opt/trn_rl_repo")

B, C, H, W = 32, 1024, 56, 56
HW = H * W                      # 3136
N_CORES = 8
B_LOC = B // N_CORES            # 4 batch items per core
ROWS = B_LOC * C                # 4096 (b, c) rows per core
RPP = 4                         # DRAM rows packed per SBUF partition
FREE = HW * RPP                 # elems per partition per tile
NT = ROWS // (128 * RPP)        # tiles per core
IO_BUFS = 3
STORE_LAG = 2
ABLATION_VALUE = 1.0e7

_CACHE: dict = {}


def _build_nc():
    import concourse.bass as bass
    import concourse.mybir as mybir
    import concourse.tile as tile
    from concourse import bacc
    from concourse.masks import make_identity
    from concourse.tile_rust import add_dep_helper

    # Bacc (not raw Bass): bacc.compile() runs the lowering passes that
    # legalize multi-semaphore waits on DMA instructions — raw Bass BIR
    # with >1 sync wait per DMA fails walrus codegen ("Too many sync
    # wait commands").
    nc = bacc.Bacc(
        "TRN2",
        target_bir_lowering=False,
        debug=False,
        num_devices=N_CORES,
    )
    f32 = mybir.dt.float32
    i32 = mybir.dt.int32

    x = nc.declare_dram_parameter("x", [ROWS, HW], f32, isOutput=False)
    # per-core indices: int64 little-endian words -> [B_LOC, 2] int32, [:, 0] = low word
    idx = nc.declare_dram_parameter("idx", [B_LOC, 2], i32, isOutput=False)
    # per-core step values (global batch index + 1) as f32
    steps = nc.declare_dram_parameter("steps", [1, B_LOC], f32, isOutput=False)
    out = nc.declare_dram_parameter("out", [ROWS, HW], f32, isOutput=True)

    xv = x.rearrange("(n p r) f -> n p (r f)", p=128, r=RPP)
    ov = out.rearrange("(n p r) f -> n p (r f)", p=128, r=RPP)

    with tile.TileContext(nc) as tc:
        with (
            tc.tile_pool(name="io", bufs=IO_BUFS) as io_pool,
            tc.tile_pool(name="small", bufs=1) as sp,
            tc.tile_pool(name="psum", bufs=1, space="PSUM") as pp,
            tc.tile_pool(name="dram", bufs=1, space="DRAM") as dp,
        ):
            # Identity for the PE-transpose partition fold, built up front
            # (overlaps the copy loop).
            ident = sp.tile([128, 128], f32)
            make_identity(nc, ident[:])

            # Per-tile min into a stats column; the store is a second reader
            # of the loaded tile and runs concurrently with the reduce.
            stats = sp.tile([128, NT], f32)
            store_insts = []
            reduce_insts = []
            for t in range(NT):
                tl = io_pool.tile([128, FREE], f32, tag="io", name=f"tl{t}")
                nc.sync.dma_start(out=tl[:], in_=xv[t])
                r = nc.vector.tensor_reduce(
                    out=stats[:, t : t + 1],
                    in_=tl[:],
                    axis=mybir.AxisListType.X,
                    op=mybir.AluOpType.min,
                )
                reduce_insts.append(r)
                s = nc.scalar.dma_start(out=ov[t], in_=tl[:])
                store_insts.append(s)
            # Let loads (and with them the min chain + AllGather doorbell)
            # run ahead of stores: store t is held until reduce t+LAG, so the
            # trailing stores overlap the ~37us AllGather instead of the
            # doorbell waiting on store bandwidth.  The edge targets the DVE
            # engine semaphore (reduces), not a DMA completion lane — a
            # store-waits-on-later-load edge can share a DMA semaphore lane
            # with its own target and deadlock.
            for t in range(NT):
                lag = min(t + STORE_LAG, NT - 1)
                if lag > t:
                    add_dep_helper(
                        store_insts[t].ins,
                        reduce_insts[lag].ins,
                        reason="delay store behind later reduce",
                    )

            # ---- fold the partition axis: PE transpose (via identity
            # matmul) puts the 128 per-partition minima into one PSUM row —
            # no DRAM round-trip, PE is otherwise idle.
            pmin = sp.tile([128, 1], f32)
            nc.vector.tensor_reduce(
                out=pmin[:, 0:1],
                in_=stats[:, :],
                axis=mybir.AxisListType.X,
                op=mybir.AluOpType.min,
            )
            pt = pp.tile([128, 128], f32)
            nc.tensor.transpose(
                out=pt[:, :],
                in_=pmin[:, 0:1].to_broadcast([128, 128]),
                identity=ident[:, :],
            )
            m_loc = sp.tile([1, 1], f32)
            nc.vector.tensor_reduce(
                out=m_loc[0:1, 0:1],
                in_=pt[0:1, :],
                axis=mybir.AxisListType.X,
                op=mybir.AluOpType.min,
            )

            # ---- exchange local minima (AllGather over 8 cores) ----
            cc_in = dp.tile([1, 1], f32)
            cc_out = dp.tile([1, N_CORES], f32, addr_space="Shared")
            nc.sync.dma_start(out=cc_in[:], in_=m_loc[0:1, 0:1])
            cc = nc.gpsimd.collective_compute(
                "AllGather",
                mybir.AluOpType.bypass,
                ins=[cc_in[:]],
                outs=[cc_out[:]],
                replica_groups=[list(range(N_CORES))],
            )
            allm = sp.tile([1, N_CORES], f32)
            ld = nc.sync.dma_start(out=allm[0:1, :], in_=cc_out[:])
            add_dep_helper(ld.ins, cc.ins, reason="read AG output after collective")

            m0 = sp.tile([1, 1], f32)
            nc.vector.tensor_reduce(
                out=m0[0:1, 0:1],
                in_=allm[0:1, :],
                axis=mybir.AxisListType.X,
                op=mybir.AluOpType.min,
            )

            # ---- vals[i] = (m0 - steps[i]*1e7) * (m0 != 0) ----
            steps_sb = sp.tile([1, B_LOC], f32)
            nc.sync.dma_start(out=steps_sb[0:1, :], in_=steps[:])
            vals = sp.tile([1, B_LOC], f32)
            nc.vector.tensor_scalar_mul(vals[0:1, :], steps_sb[0:1, :], -ABLATION_VALUE)
            nc.vector.tensor_scalar_add(vals[0:1, :], vals[0:1, :], m0[0:1, 0:1])
            nzmask = sp.tile([1, 1], f32)
            nc.vector.tensor_scalar(
                nzmask[0:1, 0:1],
                m0[0:1, 0:1],
                0.0,
                None,
                mybir.AluOpType.not_equal,
            )
            nc.vector.tensor_scalar_mul(vals[0:1, :], vals[0:1, :], nzmask[0:1, 0:1])

            # ---- move vals to one-per-partition layout, broadcast to row width ----
            vals_d = dp.tile([B_LOC, 1], f32)
            nc.sync.dma_start(
                out=vals_d[:].rearrange("a b -> b a"), in_=vals[0:1, :]
            )
            vals_p = sp.tile([B_LOC, 1], f32)
            nc.sync.dma_start(out=vals_p[:, 0:1], in_=vals_d[:])
            vrow = sp.tile([B_LOC, HW], f32)
            nc.vector.tensor_copy(
                out=vrow[:, :], in_=vals_p[:, 0:1].to_broadcast([B_LOC, HW])
            )

            # ---- dynamic row offsets: row[i] = i*C + indices[i] ----
            idx_sb = sp.tile([B_LOC, 1], i32)
            nc.sync.dma_start(out=idx_sb[:, 0:1], in_=idx[:, 0:1])
            base = sp.tile([B_LOC, 1], i32)
            nc.gpsimd.iota(base[:, 0:1], [[1, 1]], base=0, channel_multiplier=C)
            row_off = sp.tile([B_LOC, 1], i32)
            nc.vector.tensor_add(row_off[:, 0:1], idx_sb[:, 0:1], base[:, 0:1])

            # ---- scatter: out[row[i], :] = vals[i] ----
            sc = nc.gpsimd.indirect_dma_start(
                out=out[:],
                out_offset=bass.IndirectOffsetOnAxis(ap=row_off[0:B_LOC, 0:1], axis=0),
                in_=vrow[:, :],
                in_offset=None,
            )
            # The scatter rewrites rows the copy loop also writes; make the
            # ordering explicit rather than relying on DRAM dep tracking.
            for s in store_insts:
                add_dep_helper(sc.ins, s.ins, reason="scatter after copy store")

    nc.compile()
    return nc


def _get_nc():
    if "nc" not in _CACHE:
        _CACHE["nc"] = _build_nc()
    return _CACHE["nc"]


def _make_in_maps(x: np.ndarray, indices: np.ndarray):
    x = np.asarray(x, dtype=np.float32).reshape(B, C * HW)
    idx64 = np.asarray(indices).astype(np.int64, copy=False)
    idx_words = np.ascontiguousarray(idx64).view(np.int32).reshape(B, 2)
    in_maps = []
    for c in range(N_CORES):
        lo, hi = c * B_LOC, (c + 1) * B_LOC
        in_maps.append(
            {
                "x": np.ascontiguousarray(x[lo:hi]).reshape(ROWS, HW),
                "idx": np.ascontiguousarray(idx_words[lo:hi]),
                "steps": np.arange(lo + 1, hi + 1, dtype=np.float32).reshape(
                    1, B_LOC
                ),
            }
        )
    return in_maps


def _run(x: np.ndarray, indices: np.ndarray, trace: bool = False):
    from concourse.bass_utils import run_bass_kernel_spmd

    nc = _get_nc()
    res = run_bass_kernel_spmd(
        nc, _make_in_maps(x, indices), list(range(N_CORES)), trace=trace
    )
    out = np.concatenate(
        [res.results[c]["out"].reshape(B_LOC, C, H, W) for c in range(N_CORES)],
        axis=0,
    )
    return out, res


def kernel(x: np.ndarray, indices: np.ndarray) -> np.ndarray:
    out, _ = _run(x, indices, trace=False)
    return out


# revision 20
# speedup vs baseline: 1.3500x; 1.3500x over previous
"""Trainium2 Bass kernel for nn_AblationLayer.

Reference semantics (B=32, C=1024, H=W=56):
    m0 = min(x)                              # global min over all elements
    vals[i] = 0           if m0 == 0
            = m0 - (i+1)*1e7  otherwise      # i = batch index
    out = x;  out[i, indices[i], :, :] = vals[i]

Strategy: data-parallel over batch across 8 NeuronCores (4 batch items
per core).  Each core streams its 51.4MB shard through SBUF once,
computing a running min on the Vector engine while copying the data to
the output (fused min+copy: one HBM read + one HBM write).  The eight
local minima are exchanged with a tiny AllGather; each core reduces
them to the global min, computes its 4 scatter values, and overwrites
the 4 target rows with one indirect (dynamically addressed) DMA.
"""

import sys

import numpy as np

if "/opt/trn_rl_repo" not in sys.path:
    sys.path.insert(0, "/opt/trn_rl_repo")

B, C, H, W = 32, 1024, 56, 56
HW = H * W                      # 3136
N_CORES = 8
B_LOC = B // N_CORES            # 4 batch items per core
ROWS = B_LOC * C                # 4096 (b, c) rows per core
RPP = 2                         # DRAM rows packed per SBUF partition
FREE = HW * RPP                 # 6272 elems per partition per tile
NT = ROWS // (128 * RPP)        # 16 tiles per core
IO_BUFS = 6
STORE_LAG = 4
ABLATION_VALUE = 1.0e7

_CACHE: dict = {}


def _build_nc():
    import concourse.bass as bass
    import concourse.mybir as mybir
    import concourse.tile as tile
    from concourse import bacc
    from concourse.masks import make_identity
    from concourse.tile_rust import add_dep_helper

    # Bacc (not raw Bass): bacc.compile() runs the lowering passes that
    # legalize multi-semaphore waits on DMA instructions — raw Bass BIR
    # with >1 sync wait per DMA fails walrus codegen ("Too many sync
    # wait commands").
    nc = bacc.Bacc(
        "TRN2",
        target_bir_lowering=False,
        debug=False,
        num_devices=N_CORES,
    )
    f32 = mybir.dt.float32
    i32 = mybir.dt.int32

    x = nc.declare_dram_parameter("x", [ROWS, HW], f32, isOutput=False)
    # per-core indices: int64 little-endian words -> [B_LOC, 2] int32, [:, 0] = low word
    idx = nc.declare_dram_parameter("idx", [B_LOC, 2], i32, isOutput=False)
    # per-core step values (global batch index + 1) as f32
    steps = nc.declare_dram_parameter("steps", [1, B_LOC], f32, isOutput=False)
    out = nc.declare_dram_parameter("out", [ROWS, HW], f32, isOutput=True)

    xv = x.rearrange("(n p r) f -> n p (r f)", p=128, r=RPP)
    ov = out.rearrange("(n p r) f -> n p (r f)", p=128, r=RPP)

    with tile.TileContext(nc) as tc:
        with (
            tc.tile_pool(name="io", bufs=IO_BUFS) as io_pool,
            tc.tile_pool(name="small", bufs=1) as sp,
            tc.tile_pool(name="psum", bufs=1, space="PSUM") as pp,
            tc.tile_pool(name="dram", bufs=1, space="DRAM") as dp,
        ):
            # Identity for the PE-transpose partition fold, built up front
            # (overlaps the copy loop).
            ident = sp.tile([128, 128], f32)
            make_identity(nc, ident[:])

            # Per-tile min into a stats column; the store is a second reader
            # of the loaded tile and runs concurrently with the reduce.
            stats = sp.tile([128, NT], f32)
            store_insts = []
            reduce_insts = []
            for t in range(NT):
                tl = io_pool.tile([128, FREE], f32, tag="io", name=f"tl{t}")
                nc.sync.dma_start(out=tl[:], in_=xv[t])
                r = nc.vector.tensor_reduce(
                    out=stats[:, t : t + 1],
                    in_=tl[:],
                    axis=mybir.AxisListType.X,
                    op=mybir.AluOpType.min,
                )
                reduce_insts.append(r)
                s = nc.scalar.dma_start(out=ov[t], in_=tl[:])
                store_insts.append(s)
            # Let loads (and with them the min chain + AllGather doorbell)
            # run ahead of stores: store t is held until reduce t+LAG, so the
            # trailing stores overlap the ~37us AllGather instead of the
            # doorbell waiting on store bandwidth.  The edge targets the DVE
            # engine semaphore (reduces), not a DMA completion lane — a
            # store-waits-on-later-load edge can share a DMA semaphore lane
            # with its own target and deadlock.
            for t in range(NT):
                lag = min(t + STORE_LAG, NT - 1)
                if lag > t:
                    add_dep_helper(
                        store_insts[t].ins,
                        reduce_insts[lag].ins,
                        reason="delay store behind later reduce",
                    )

            # ---- fold the partition axis: PE transpose (via identity
            # matmul) puts the 128 per-partition minima into one PSUM row —
            # no DRAM round-trip, PE is otherwise idle.
            pmin = sp.tile([128, 1], f32)
            nc.vector.tensor_reduce(
                out=pmin[:, 0:1],
                in_=stats[:, :],
                axis=mybir.AxisListType.X,
                op=mybir.AluOpType.min,
            )
            pt = pp.tile([128, 128], f32)
            nc.tensor.transpose(
                out=pt[:, :],
                in_=pmin[:, 0:1].to_broadcast([128, 128]),
                identity=ident[:, :],
            )
            m_loc = sp.tile([1, 1], f32)
            nc.vector.tensor_reduce(
                out=m_loc[0:1, 0:1],
                in_=pt[0:1, :],
                axis=mybir.AxisListType.X,
                op=mybir.AluOpType.min,
            )

            # ---- exchange local minima (AllGather over 8 cores) ----
            cc_in = dp.tile([1, 1], f32)
            cc_out = dp.tile([1, N_CORES], f32, addr_space="Shared")
            nc.sync.dma_start(out=cc_in[:], in_=m_loc[0:1, 0:1])
            cc = nc.gpsimd.collective_compute(
                "AllGather",
                mybir.AluOpType.bypass,
                ins=[cc_in[:]],
                outs=[cc_out[:]],
                replica_groups=[list(range(N_CORES))],
            )
            allm = sp.tile([1, N_CORES], f32)
            ld = nc.sync.dma_start(out=allm[0:1, :], in_=cc_out[:])
            add_dep_helper(ld.ins, cc.ins, reason="read AG output after collective")

            m0 = sp.tile([1, 1], f32)
            nc.vector.tensor_reduce(
                out=m0[0:1, 0:1],
                in_=allm[0:1, :],
                axis=mybir.AxisListType.X,
                op=mybir.AluOpType.min,
            )

            # ---- vals[i] = (m0 - steps[i]*1e7) * (m0 != 0) ----
            steps_sb = sp.tile([1, B_LOC], f32)
            nc.sync.dma_start(out=steps_sb[0:1, :], in_=steps[:])
            vals = sp.tile([1, B_LOC], f32)
            nc.vector.tensor_scalar_mul(vals[0:1, :], steps_sb[0:1, :], -ABLATION_VALUE)
            nc.vector.tensor_scalar_add(vals[0:1, :], vals[0:1, :], m0[0:1, 0:1])
            nzmask = sp.tile([1, 1], f32)
            nc.vector.tensor_scalar(
                nzmask[0:1, 0:1],
                m0[0:1, 0:1],
                0.0,
                None,
                mybir.AluOpType.not_equal,
            )
            nc.vector.tensor_scalar_mul(vals[0:1, :], vals[0:1, :], nzmask[0:1, 0:1])

            # ---- move vals to one-per-partition layout, broadcast to row width ----
            vals_d = dp.tile([B_LOC, 1], f32)
            nc.sync.dma_start(
                out=vals_d[:].rearrange("a b -> b a"), in_=vals[0:1, :]
            )
            vals_p = sp.tile([B_LOC, 1], f32)
            nc.sync.dma_start(out=vals_p[:, 0:1], in_=vals_d[:])
            vrow = sp.tile([B_LOC, HW], f32)
            nc.vector.tensor_copy(
                out=vrow[:, :], in_=vals_p[:, 0:1].to_broadcast([B_LOC, HW])
            )

            # ---- dynamic row offsets: row[i] = i*C + indices[i] ----
            idx_sb = sp.tile([B_LOC, 1], i32)
            nc.sync.dma_start(out=idx_sb[:, 0:1], in_=idx[:, 0:1])
            base = sp.tile([B_LOC, 1], i32)
            nc.gpsimd.iota(base[:, 0:1], [[1, 1]], base=0, channel_multiplier=C)
            row_off = sp.tile([B_LOC, 1], i32)
            nc.vector.tensor_add(row_off[:, 0:1], idx_sb[:, 0:1], base[:, 0:1])

            # ---- scatter: out[row[i], :] = vals[i] ----
            sc = nc.gpsimd.indirect_dma_start(
                out=out[:],
                out_offset=bass.IndirectOffsetOnAxis(ap=row_off[0:B_LOC, 0:1], axis=0),
                in_=vrow[:, :],
                in_offset=None,
            )
            # The scatter rewrites rows the copy loop also writes; make the
            # ordering explicit rather than relying on DRAM dep tracking.
            for s in store_insts:
                add_dep_helper(sc.ins, s.ins, reason="scatter after copy store")

    nc.compile()
    return nc


def _get_nc():
    if "nc" not in _CACHE:
        _CACHE["nc"] = _build_nc()
    return _CACHE["nc"]


def _make_in_maps(x: np.ndarray, indices: np.ndarray):
    x = np.asarray(x, dtype=np.float32).reshape(B, C * HW)
    idx64 = np.asarray(indices).astype(np.int64, copy=False)
    idx_words = np.ascontiguousarray(idx64).view(np.int32).reshape(B, 2)
    in_maps = []
    for c in range(N_CORES):
        lo, hi = c * B_LOC, (c + 1) * B_LOC
        in_maps.append(
            {
                "x": np.ascontiguousarray(x[lo:hi]).reshape(ROWS, HW),
                "idx": np.ascontiguousarray(idx_words[lo:hi]),
                "steps": np.arange(lo + 1, hi + 1, dtype=np.float32).reshape(
                    1, B_LOC
                ),
            }
        )
    return in_maps


def _run(x: np.ndarray, indices: np.ndarray, trace: bool = False):
    from concourse.bass_utils import run_bass_kernel_spmd

    nc = _get_nc()
    res = run_bass_kernel_spmd(
        nc, _make_in_maps(x, indices), list(range(N_CORES)), trace=trace
    )
    out = np.concatenate(
        [res.results[c]["out"].reshape(B_LOC, C, H, W) for c in range(N_CORES)],
        axis=0,
    )
    return out, res


def kernel(x: np.ndarray, indices: np.ndarray) -> np.ndarray:
    out, _ = _run(x, indices, trace=False)
    return out


# revision 25
# speedup vs baseline: 1.3970x; 1.0349x over previous
"""Trainium2 Bass kernel for nn_AblationLayer.

Reference semantics (B=32, C=1024, H=W=56):
    m0 = min(x)                              # global min over all elements
    vals[i] = 0           if m0 == 0
            = m0 - (i+1)*1e7  otherwise      # i = batch index
    out = x;  out[i, indices[i], :, :] = vals[i]

Strategy: data-parallel over batch across 8 NeuronCores (4 batch items
per core).  Each core streams its 51.4MB shard through SBUF once,
computing a running min on the Vector engine while copying the data to
the output (fused min+copy: one HBM read + one HBM write).  The eight
local minima are exchanged with a tiny AllGather; each core reduces
them to the global min, computes its 4 scatter values, and overwrites
the 4 target rows with one indirect (dynamically addressed) DMA.
"""

import sys

import numpy as np

if "/opt/trn_rl_repo" not in sys.path:
    sys.path.insert(0, "/opt/trn_rl_repo")

B, C, H, W = 32, 1024, 56, 56
HW = H * W                      # 3136
N_CORES = 8
B_LOC = B // N_CORES            # 4 batch items per core
ROWS = B_LOC * C                # 4096 (b, c) rows per core
RPP = 2                         # DRAM rows packed per SBUF partition
FREE = HW * RPP                 # 6272 elems per partition per tile
NT = ROWS // (128 * RPP)        # 16 tiles per core
IO_BUFS = 7
STORE_LAG = 5
ABLATION_VALUE = 1.0e7

_CACHE: dict = {}


def _build_nc():
    import concourse.bass as bass
    import concourse.mybir as mybir
    import concourse.tile as tile
    from concourse import bacc
    from concourse.masks import make_identity
    from concourse.tile_rust import add_dep_helper

    # Bacc (not raw Bass): bacc.compile() runs the lowering passes that
    # legalize multi-semaphore waits on DMA instructions — raw Bass BIR
    # with >1 sync wait per DMA fails walrus codegen ("Too many sync
    # wait commands").
    nc = bacc.Bacc(
        "TRN2",
        target_bir_lowering=False,
        debug=False,
        num_devices=N_CORES,
    )
    f32 = mybir.dt.float32
    i32 = mybir.dt.int32

    x = nc.declare_dram_parameter("x", [ROWS, HW], f32, isOutput=False)
    # per-core indices: int64 little-endian words -> [B_LOC, 2] int32, [:, 0] = low word
    idx = nc.declare_dram_parameter("idx", [B_LOC, 2], i32, isOutput=False)
    # per-core step values (global batch index + 1) as f32, one per partition
    steps = nc.declare_dram_parameter("steps", [B_LOC, 1], f32, isOutput=False)
    out = nc.declare_dram_parameter("out", [ROWS, HW], f32, isOutput=True)

    xv = x.rearrange("(n p r) f -> n p (r f)", p=128, r=RPP)
    ov = out.rearrange("(n p r) f -> n p (r f)", p=128, r=RPP)

    with tile.TileContext(nc) as tc:
        with (
            tc.tile_pool(name="io", bufs=IO_BUFS) as io_pool,
            tc.tile_pool(name="small", bufs=1) as sp,
            tc.tile_pool(name="psum", bufs=1, space="PSUM") as pp,
            tc.tile_pool(name="dram", bufs=1, space="DRAM") as dp,
        ):
            # Identity for the PE-transpose partition fold, a ones row for
            # the PE broadcast of the gathered minima, and the pre-scaled
            # steps — all built up front (overlap the copy loop).
            ident = sp.tile([128, 128], f32)
            make_identity(nc, ident[:])
            ones14 = sp.tile([1, B_LOC], f32)
            nc.gpsimd.memset(ones14[0:1, :], 1.0)
            steps_sb = sp.tile([B_LOC, 1], f32)
            nc.sync.dma_start(out=steps_sb[:, 0:1], in_=steps[:])
            sm = sp.tile([B_LOC, 1], f32)
            nc.vector.tensor_scalar_mul(
                sm[:, 0:1], steps_sb[:, 0:1], -ABLATION_VALUE
            )

            # Per-tile min into a stats column; the store is a second reader
            # of the loaded tile and runs concurrently with the reduce.
            stats = sp.tile([128, NT], f32)
            store_insts = []
            reduce_insts = []
            for t in range(NT):
                tl = io_pool.tile([128, FREE], f32, tag="io", name=f"tl{t}")
                nc.sync.dma_start(out=tl[:], in_=xv[t])
                r = nc.vector.tensor_reduce(
                    out=stats[:, t : t + 1],
                    in_=tl[:],
                    axis=mybir.AxisListType.X,
                    op=mybir.AluOpType.min,
                )
                reduce_insts.append(r)
                s = nc.scalar.dma_start(out=ov[t], in_=tl[:])
                store_insts.append(s)
            # Let loads (and with them the min chain + AllGather doorbell)
            # run ahead of stores: store t is held until reduce t+LAG, so the
            # trailing stores overlap the ~37us AllGather instead of the
            # doorbell waiting on store bandwidth.  The edge targets the DVE
            # engine semaphore (reduces), not a DMA completion lane — a
            # store-waits-on-later-load edge can share a DMA semaphore lane
            # with its own target and deadlock.
            for t in range(NT):
                lag = min(t + STORE_LAG, NT - 1)
                if lag > t:
                    add_dep_helper(
                        store_insts[t].ins,
                        reduce_insts[lag].ins,
                        reason="delay store behind later reduce",
                    )

            # ---- fold the partition axis: PE transpose (via identity
            # matmul) puts the 128 per-partition minima into one PSUM row —
            # no DRAM round-trip, PE is otherwise idle.
            pmin = sp.tile([128, 1], f32)
            nc.vector.tensor_reduce(
                out=pmin[:, 0:1],
                in_=stats[:, :],
                axis=mybir.AxisListType.X,
                op=mybir.AluOpType.min,
            )
            pt = pp.tile([128, 128], f32)
            nc.tensor.transpose(
                out=pt[:, :],
                in_=pmin[:, 0:1].to_broadcast([128, 128]),
                identity=ident[:, :],
            )
            m_loc = sp.tile([1, 1], f32)
            nc.vector.tensor_reduce(
                out=m_loc[0:1, 0:1],
                in_=pt[0:1, :],
                axis=mybir.AxisListType.X,
                op=mybir.AluOpType.min,
            )

            # ---- exchange local minima (AllGather over 8 cores); the
            # staging DMA runs on gpsimd so the collective doorbell follows
            # in program order with no cross-engine hop.
            cc_in = dp.tile([1, 1], f32)
            cc_out = dp.tile([1, N_CORES], f32, addr_space="Shared")
            nc.gpsimd.dma_start(out=cc_in[:], in_=m_loc[0:1, 0:1])
            cc = nc.gpsimd.collective_compute(
                "AllGather",
                mybir.AluOpType.bypass,
                ins=[cc_in[:]],
                outs=[cc_out[:]],
                replica_groups=[list(range(N_CORES))],
            )
            allm = sp.tile([1, N_CORES], f32)
            ld = nc.sync.dma_start(out=allm[0:1, :], in_=cc_out[:])
            add_dep_helper(ld.ins, cc.ins, reason="read AG output after collective")

            # ---- m0 on B_LOC partitions: PE outer product replicates the
            # gathered row (ones[1,B]ᵀ @ allm[1,8] -> psum[B,8]), then a
            # per-partition min — no DRAM bounce for the broadcast.
            mm = pp.tile([B_LOC, N_CORES], f32)
            nc.tensor.matmul(
                mm[:, :], ones14[0:1, :], allm[0:1, :], start=True, stop=True
            )
            m0_4 = sp.tile([B_LOC, 1], f32)
            nc.vector.tensor_reduce(
                out=m0_4[:, 0:1],
                in_=mm[:, :],
                axis=mybir.AxisListType.X,
                op=mybir.AluOpType.min,
            )

            # ---- vals[i] = (m0 - steps[i]*1e7) * (m0 != 0), per partition
            nz4 = sp.tile([B_LOC, 1], f32)
            nc.vector.tensor_scalar(
                nz4[:, 0:1],
                m0_4[:, 0:1],
                0.0,
                None,
                mybir.AluOpType.not_equal,
            )
            vals_p = sp.tile([B_LOC, 1], f32)
            nc.vector.tensor_add(vals_p[:, 0:1], sm[:, 0:1], m0_4[:, 0:1])
            nc.vector.tensor_mul(vals_p[:, 0:1], vals_p[:, 0:1], nz4[:, 0:1])
            vrow = sp.tile([B_LOC, HW], f32)
            nc.vector.tensor_copy(
                out=vrow[:, :], in_=vals_p[:, 0:1].to_broadcast([B_LOC, HW])
            )

            # ---- dynamic row offsets: row[i] = i*C + indices[i] ----
            idx_sb = sp.tile([B_LOC, 1], i32)
            nc.sync.dma_start(out=idx_sb[:, 0:1], in_=idx[:, 0:1])
            base = sp.tile([B_LOC, 1], i32)
            nc.gpsimd.iota(base[:, 0:1], [[1, 1]], base=0, channel_multiplier=C)
            row_off = sp.tile([B_LOC, 1], i32)
            nc.vector.tensor_add(row_off[:, 0:1], idx_sb[:, 0:1], base[:, 0:1])

            # ---- scatter: out[row[i], :] = vals[i] ----
            sc = nc.gpsimd.indirect_dma_start(
                out=out[:],
                out_offset=bass.IndirectOffsetOnAxis(ap=row_off[0:B_LOC, 0:1], axis=0),
                in_=vrow[:, :],
                in_offset=None,
            )
            # The scatter rewrites rows the copy loop also writes; make the
            # ordering explicit rather than relying on DRAM dep tracking.
            for s in store_insts:
                add_dep_helper(sc.ins, s.ins, reason="scatter after copy store")

    nc.compile()
    return nc


def _get_nc():
    if "nc" not in _CACHE:
        _CACHE["nc"] = _build_nc()
    return _CACHE["nc"]


def _make_in_maps(x: np.ndarray, indices: np.ndarray):
    x = np.asarray(x, dtype=np.float32).reshape(B, C * HW)
    idx64 = np.asarray(indices).astype(np.int64, copy=False)
    idx_words = np.ascontiguousarray(idx64).view(np.int32).reshape(B, 2)
    in_maps = []
    for c in range(N_CORES):
        lo, hi = c * B_LOC, (c + 1) * B_LOC
        in_maps.append(
            {
                "x": np.ascontiguousarray(x[lo:hi]).reshape(ROWS, HW),
                "idx": np.ascontiguousarray(idx_words[lo:hi]),
                "steps": np.arange(lo + 1, hi + 1, dtype=np.float32).reshape(
                    B_LOC, 1
                ),
            }
        )
    return in_maps


def _run(x: np.ndarray, indices: np.ndarray, trace: bool = False):
    from concourse.bass_utils import run_bass_kernel_spmd

    nc = _get_nc()
    res = run_bass_kernel_spmd(
        nc, _make_in_maps(x, indices), list(range(N_CORES)), trace=trace
    )
    out = np.concatenate(
        [res.results[c]["out"].reshape(B_LOC, C, H, W) for c in range(N_CORES)],
        axis=0,
    )
    return out, res


def kernel(x: np.ndarray, indices: np.ndarray) -> np.ndarray:
    out, _ = _run(x, indices, trace=False)
    return out


# revision 26
# speedup vs baseline: 1.4117x; 1.0105x over previous
"""Trainium2 Bass kernel for nn_AblationLayer.

Reference semantics (B=32, C=1024, H=W=56):
    m0 = min(x)                              # global min over all elements
    vals[i] = 0           if m0 == 0
            = m0 - (i+1)*1e7  otherwise      # i = batch index
    out = x;  out[i, indices[i], :, :] = vals[i]

Strategy: data-parallel over batch across 8 NeuronCores (4 batch items
per core).  Each core streams its 51.4MB shard through SBUF once,
computing a running min on the Vector engine while copying the data to
the output (fused min+copy: one HBM read + one HBM write).  The eight
local minima are exchanged with a tiny AllGather; each core reduces
them to the global min, computes its 4 scatter values, and overwrites
the 4 target rows with one indirect (dynamically addressed) DMA.
"""

import sys

import numpy as np

if "/opt/trn_rl_repo" not in sys.path:
    sys.path.insert(0, "/opt/trn_rl_repo")

B, C, H, W = 32, 1024, 56, 56
HW = H * W                      # 3136
N_CORES = 8
B_LOC = B // N_CORES            # 4 batch items per core
ROWS = B_LOC * C                # 4096 (b, c) rows per core
RPP = 2                         # DRAM rows packed per SBUF partition
FREE = HW * RPP                 # 6272 elems per partition per tile
NT = ROWS // (128 * RPP)        # 16 tiles per core
IO_BUFS = 7
STORE_LAG = 5
ABLATION_VALUE = 1.0e7

_CACHE: dict = {}


def _build_nc():
    import concourse.bass as bass
    import concourse.mybir as mybir
    import concourse.tile as tile
    from concourse import bacc
    from concourse.masks import make_identity
    from concourse.tile_rust import add_dep_helper

    # Bacc (not raw Bass): bacc.compile() runs the lowering passes that
    # legalize multi-semaphore waits on DMA instructions — raw Bass BIR
    # with >1 sync wait per DMA fails walrus codegen ("Too many sync
    # wait commands").
    nc = bacc.Bacc(
        "TRN2",
        target_bir_lowering=False,
        debug=False,
        num_devices=N_CORES,
    )
    f32 = mybir.dt.float32
    i32 = mybir.dt.int32

    x = nc.declare_dram_parameter("x", [ROWS, HW], f32, isOutput=False)
    # per-core indices: int64 little-endian words -> [B_LOC, 2] int32, [:, 0] = low word
    idx = nc.declare_dram_parameter("idx", [B_LOC, 2], i32, isOutput=False)
    # per-core step values (global batch index + 1) as f32, one per partition
    steps = nc.declare_dram_parameter("steps", [B_LOC, 1], f32, isOutput=False)
    out = nc.declare_dram_parameter("out", [ROWS, HW], f32, isOutput=True)

    xv = x.rearrange("(n p r) f -> n p (r f)", p=128, r=RPP)
    ov = out.rearrange("(n p r) f -> n p (r f)", p=128, r=RPP)

    with tile.TileContext(nc) as tc:
        with (
            tc.tile_pool(name="io", bufs=IO_BUFS) as io_pool,
            tc.tile_pool(name="small", bufs=1) as sp,
            tc.tile_pool(name="psum", bufs=1, space="PSUM") as pp,
            tc.tile_pool(name="dram", bufs=1, space="DRAM") as dp,
        ):
            # Identity for the PE-transpose partition fold, a ones row for
            # the PE broadcast of the gathered minima, and the pre-scaled
            # steps — all built up front (overlap the copy loop).
            ident = sp.tile([128, 128], f32)
            make_identity(nc, ident[:])
            ones14 = sp.tile([1, B_LOC], f32)
            nc.gpsimd.memset(ones14[0:1, :], 1.0)
            steps_sb = sp.tile([B_LOC, 1], f32)
            nc.sync.dma_start(out=steps_sb[:, 0:1], in_=steps[:])
            sm = sp.tile([B_LOC, 1], f32)
            nc.vector.tensor_scalar_mul(
                sm[:, 0:1], steps_sb[:, 0:1], -ABLATION_VALUE
            )

            # Per-tile min into a stats column; the store is a second reader
            # of the loaded tile and runs concurrently with the reduce.
            stats = sp.tile([128, NT], f32)
            store_insts = []
            reduce_insts = []
            for t in range(NT):
                tl = io_pool.tile([128, FREE], f32, tag="io", name=f"tl{t}")
                nc.sync.dma_start(out=tl[:], in_=xv[t])
                r = nc.vector.tensor_reduce(
                    out=stats[:, t : t + 1],
                    in_=tl[:],
                    axis=mybir.AxisListType.X,
                    op=mybir.AluOpType.min,
                )
                reduce_insts.append(r)
                s = nc.scalar.dma_start(out=ov[t], in_=tl[:])
                store_insts.append(s)
            # Let loads (and with them the min chain + AllGather doorbell)
            # run ahead of stores: store t is held until reduce t+LAG, so the
            # trailing stores overlap the ~37us AllGather instead of the
            # doorbell waiting on store bandwidth.  The edge targets the DVE
            # engine semaphore (reduces), not a DMA completion lane — a
            # store-waits-on-later-load edge can share a DMA semaphore lane
            # with its own target and deadlock.
            for t in range(NT):
                lag = min(t + STORE_LAG, NT - 1)
                if lag > t:
                    add_dep_helper(
                        store_insts[t].ins,
                        reduce_insts[lag].ins,
                        reason="delay store behind later reduce",
                    )

            # ---- fold the partition axis: PE transpose (via identity
            # matmul) puts the 128 per-partition minima into one PSUM row —
            # no DRAM round-trip, PE is otherwise idle.
            pmin = sp.tile([128, 1], f32)
            nc.vector.tensor_reduce(
                out=pmin[:, 0:1],
                in_=stats[:, :],
                axis=mybir.AxisListType.X,
                op=mybir.AluOpType.min,
            )
            pt = pp.tile([128, 128], f32)
            nc.tensor.transpose(
                out=pt[:, :],
                in_=pmin[:, 0:1].to_broadcast([128, 128]),
                identity=ident[:, :],
            )
            m_loc = sp.tile([1, 1], f32)
            nc.vector.tensor_reduce(
                out=m_loc[0:1, 0:1],
                in_=pt[0:1, :],
                axis=mybir.AxisListType.X,
                op=mybir.AluOpType.min,
            )

            # ---- exchange local minima (AllGather over 8 cores); the
            # staging DMA runs on gpsimd so the collective doorbell follows
            # in program order with no cross-engine hop.
            cc_in = dp.tile([1, 1], f32)
            cc_out = dp.tile([1, N_CORES], f32, addr_space="Shared")
            nc.gpsimd.dma_start(out=cc_in[:], in_=m_loc[0:1, 0:1])
            cc = nc.gpsimd.collective_compute(
                "AllGather",
                mybir.AluOpType.bypass,
                ins=[cc_in[:]],
                outs=[cc_out[:]],
                replica_groups=[list(range(N_CORES))],
            )
            # Read the gathered row from the same engine that triggered the
            # collective — no cross-engine semaphore hop on the critical path.
            allm = sp.tile([1, N_CORES], f32)
            ld = nc.gpsimd.dma_start(out=allm[0:1, :], in_=cc_out[:])
            add_dep_helper(ld.ins, cc.ins, reason="read AG output after collective")

            # ---- m0 on B_LOC partitions: PE outer product replicates the
            # gathered row (ones[1,B]ᵀ @ allm[1,8] -> psum[B,8]), then a
            # per-partition min — no DRAM bounce for the broadcast.
            mm = pp.tile([B_LOC, N_CORES], f32)
            nc.tensor.matmul(
                mm[:, :], ones14[0:1, :], allm[0:1, :], start=True, stop=True
            )
            m0_4 = sp.tile([B_LOC, 1], f32)
            nc.vector.tensor_reduce(
                out=m0_4[:, 0:1],
                in_=mm[:, :],
                axis=mybir.AxisListType.X,
                op=mybir.AluOpType.min,
            )

            # ---- vals[i] = (m0 - steps[i]*1e7) * (m0 != 0), per partition
            nz4 = sp.tile([B_LOC, 1], f32)
            nc.vector.tensor_scalar(
                nz4[:, 0:1],
                m0_4[:, 0:1],
                0.0,
                None,
                mybir.AluOpType.not_equal,
            )
            vals_p = sp.tile([B_LOC, 1], f32)
            nc.vector.tensor_add(vals_p[:, 0:1], sm[:, 0:1], m0_4[:, 0:1])
            nc.vector.tensor_mul(vals_p[:, 0:1], vals_p[:, 0:1], nz4[:, 0:1])
            vrow = sp.tile([B_LOC, HW], f32)
            nc.vector.tensor_copy(
                out=vrow[:, :], in_=vals_p[:, 0:1].to_broadcast([B_LOC, HW])
            )

            # ---- dynamic row offsets: row[i] = i*C + indices[i] ----
            idx_sb = sp.tile([B_LOC, 1], i32)
            nc.sync.dma_start(out=idx_sb[:, 0:1], in_=idx[:, 0:1])
            base = sp.tile([B_LOC, 1], i32)
            nc.gpsimd.iota(base[:, 0:1], [[1, 1]], base=0, channel_multiplier=C)
            row_off = sp.tile([B_LOC, 1], i32)
            nc.vector.tensor_add(row_off[:, 0:1], idx_sb[:, 0:1], base[:, 0:1])

            # ---- scatter: out[row[i], :] = vals[i] ----
            sc = nc.gpsimd.indirect_dma_start(
                out=out[:],
                out_offset=bass.IndirectOffsetOnAxis(ap=row_off[0:B_LOC, 0:1], axis=0),
                in_=vrow[:, :],
                in_offset=None,
            )
            # The scatter rewrites rows the copy loop also writes; make the
            # ordering explicit rather than relying on DRAM dep tracking.
            for s in store_insts:
                add_dep_helper(sc.ins, s.ins, reason="scatter after copy store")

    nc.compile()
    return nc


def _get_nc():
    if "nc" not in _CACHE:
        _CACHE["nc"] = _build_nc()
    return _CACHE["nc"]


def _make_in_maps(x: np.ndarray, indices: np.ndarray):
    x = np.asarray(x, dtype=np.float32).reshape(B, C * HW)
    idx64 = np.asarray(indices).astype(np.int64, copy=False)
    idx_words = np.ascontiguousarray(idx64).view(np.int32).reshape(B, 2)
    in_maps = []
    for c in range(N_CORES):
        lo, hi = c * B_LOC, (c + 1) * B_LOC
        in_maps.append(
            {
                "x": np.ascontiguousarray(x[lo:hi]).reshape(ROWS, HW),
                "idx": np.ascontiguousarray(idx_words[lo:hi]),
                "steps": np.arange(lo + 1, hi + 1, dtype=np.float32).reshape(
                    B_LOC, 1
                ),
            }
        )
    return in_maps


def _run(x: np.ndarray, indices: np.ndarray, trace: bool = False):
    from concourse.bass_utils import run_bass_kernel_spmd

    nc = _get_nc()
    res = run_bass_kernel_spmd(
        nc, _make_in_maps(x, indices), list(range(N_CORES)), trace=trace
    )
    out = np.concatenate(
        [res.results[c]["out"].reshape(B_LOC, C, H, W) for c in range(N_CORES)],
        axis=0,
    )
    return out, res


def kernel(x: np.ndarray, indices: np.ndarray) -> np.ndarray:
    out, _ = _run(x, indices, trace=False)
    return out
